# revision 36
# baseline (speedup 1.0000x reference)
"""Single-head attention (B=4, S=2048, D=1024) on 8 Trainium2 NeuronCores.

Sharding: core c handles batch b = c//2, query half h = c%2 (1024 queries).
V for the full sequence is obtained by each core projecting its own half and
exchanging halves with its pair via AllGather.

Math notes (exact rewrites of the reference):
  - scores = (x@Wq^T + bq)(x@Wk^T + bk)^T / 32. Softmax is invariant to
    per-row constants, so the bk terms drop. The rest factors as
      scores = (x @ A + r) @ x^T / 32,  A = Wq^T @ Wk,  r = bq @ Wk,
    with A, r precomputed on host in fp32 (weight prep). This removes the
    K projection from the device entirely.
  - attn rows sum to 1, so the V bias bv is a constant additive term on
    the output: out = attn @ V_nobias + bv.
  - softmax is computed without max-subtraction: |scores/32| < ~4 for
    this problem (checked host-side), exp() is well-conditioned there.

Device pipeline per core (all matmul operands bf16, PSUM accumulation fp32):
  Phase V:  V[s,e] = xq^T @ WvT for own half (PE), exported per s-tile,
            pair AllGather -> full V in SBUF.
  Phase P:  Pt[d,q] = A^T-applied projection (+r via ACT bias) -> bf16.
  Phase B:  per 512-query chunk: scoresT[k,q] = xt^T(slice) @ Pt (PE),
            exp(s/32) on ACT straight into SBUF (this IS the attn@V lhsT —
            no transposes); per 128-query tile: den[q] via ones-matmul
            accumulation, out accум = attnT^T @ V; ACT applies 1/den,
            DVE adds bv; DMA out.
"""

import numpy as np
import ml_dtypes

from contextlib import ExitStack

import concourse.bass as bass
import concourse.mybir as mybir
import concourse.tile as tile
from concourse import bacc

BF16 = mybir.dt.bfloat16
F32 = mybir.dt.float32
NPBF16 = ml_dtypes.bfloat16

B, S, D = 4, 2048, 1024
NCORES = 8
SQ = S // 2            # queries per core
P = 128                # partitions
NDT = D // P           # 8 d-tiles (feature dim)
NST = S // P           # 16 key tiles
NQT = SQ // P          # 8 query tiles per core
NQC = SQ // 512        # 2 query chunks of 512
NEC = D // 512         # 2 embed chunks of 512
SCALE = 1.0 / 32.0     # 1/sqrt(D)

AF = mybir.ActivationFunctionType

_PROGRAM = None


def _build_program():
    nc = bacc.Bacc(
        "TRN2", target_bir_lowering=False, debug=False, num_devices=NCORES
    )
    # all inputs host-packed partition-tiled [128, n]: one fully-contiguous
    # (16KB+ rows) DMA each — trigger issue (~700ns each) and small-packet
    # overhead dominated the startup otherwise
    xt_d = nc.dram_tensor("xt", [P, NDT * S], BF16, kind="ExternalInput")
    xq_d = nc.dram_tensor("xq", [P, NQT * NDT * P], BF16, kind="ExternalInput")
    a_d = nc.dram_tensor("a", [P, NDT * D], BF16, kind="ExternalInput")
    wv_d = nc.dram_tensor("wv", [P, NEC * NDT * 512], BF16, kind="ExternalInput")
    r_d = nc.dram_tensor("r", [P, NDT], F32, kind="ExternalInput")
    bv_d = nc.dram_tensor("bv", [1, D], F32, kind="ExternalInput")
    out_d = nc.dram_tensor("out", [SQ, D], F32, kind="ExternalOutput")

    with tile.TileContext(nc) as tc, ExitStack() as ctx:
        consts = ctx.enter_context(tc.tile_pool(name="consts", bufs=1))
        xpool = ctx.enter_context(tc.tile_pool(name="xpool", bufs=1))
        wpool = ctx.enter_context(tc.tile_pool(name="wpool", bufs=1))
        stage = ctx.enter_context(tc.tile_pool(name="stage", bufs=1))
        proj = ctx.enter_context(tc.tile_pool(name="proj", bufs=1))
        epool = ctx.enter_context(tc.tile_pool(name="epool", bufs=1))
        bpool = ctx.enter_context(tc.tile_pool(name="bpool", bufs=1))
        dpool = ctx.enter_context(tc.tile_pool(name="dpool", bufs=1, space="DRAM"))
        ps = ctx.enter_context(tc.tile_pool(name="ps", bufs=5, space="PSUM"))
        pst = ctx.enter_context(tc.tile_pool(name="pst", bufs=3, space="PSUM"))

        # --- PE warm-up: dummy matmuls on a zeroed tile keep the PE busy
        # (and the HAM clock-gate warming) while the first inputs land ---
        warm = consts.tile([P, 640], BF16)
        nc.vector.memset(warm[:], 0.0)

        # tiny warm-up collective: absorbs any one-time CC channel setup
        # latency before the real V exchange
        ccw_in = dpool.tile([1, 256], BF16, tag="ccw_in")
        ccw_out = dpool.tile([2, 256], BF16, tag="ccw_out")
        nc.gpsimd.dma_start(out=ccw_in[:], in_=warm[0:1, 0:256])
        nc.gpsimd.collective_compute(
            "AllGather", mybir.AluOpType.bypass,
            replica_groups=[[2 * i, 2 * i + 1] for i in range(NCORES // 2)],
            ins=[ccw_in[:]], outs=[ccw_out[:]],
        )
        for _ in range(14):
            wps = ps.tile([P, 512], F32, name="psum")
            nc.tensor.matmul(
                wps[:], lhsT=warm[:, 512:640], rhs=warm[:, 0:512],
                start=True, stop=True,
            )

        # --- input loads: DMA trigger issue costs ~700ns per dma_start on
        # the issuing engine's queue, so round-robin the triggers over the
        # HWDGE-capable engines, first-needed data first ---
        # keep the scalar engine OUT of trigger duty: it must drain the
        # first V PSUMs while inputs are still streaming
        trig = [nc.sync, nc.gpsimd]
        _t = [0]

        def dma(out, in_):
            trig[_t[0] % len(trig)].dma_start(out=out, in_=in_)
            _t[0] += 1

        # first-needed first: V(st, ec0) chains need wv-ec0 + the st'th 256KB
        # block of the st-major-packed xq. DMA-completion semaphores are per
        # dma_start, so xq is split into 2-st-block pieces: the first V chain
        # only waits on wv-ec0 + piece 0, not the whole 2MB
        # interleave wv-ec0 (2-dt 256KB pieces) with per-st 256KB xq pieces:
        # the first V matmul needs only wv piece 0 + xq piece 0 = 512KB
        wv_sb = wpool.tile([P, NEC * NDT * 512], BF16, tag="w")
        xq_sb = xpool.tile([P, NQT * NDT * P], BF16)
        XB = NQT * NDT * P // 8
        dma(wv_sb[:, 0:1024], wv_d[:, 0:1024])
        dma(xq_sb[:, 0:XB], xq_d[:, 0:XB])
        for b_ in range(1, 4):
            dma(wv_sb[:, b_ * 1024:(b_ + 1) * 1024], wv_d[:, b_ * 1024:(b_ + 1) * 1024])
            dma(xq_sb[:, b_ * XB:(b_ + 1) * XB], xq_d[:, b_ * XB:(b_ + 1) * XB])
        for b_ in range(4, 8):
            dma(xq_sb[:, b_ * XB:(b_ + 1) * XB], xq_d[:, b_ * XB:(b_ + 1) * XB])
        dma(wv_sb[:, NDT * 512:2 * NDT * 512], wv_d[:, NDT * 512:2 * NDT * 512])
        # a/xt are triggered from the scalar queue mid-V-phase (below): the
        # DMA queue shares bandwidth round-robin across active transfers, so
        # issuing them now would delay the startup-critical wv/xq pieces
        a_sb = xpool.tile([P, NDT * D], BF16)
        xt_sb = xpool.tile([P, NDT * S], BF16)
        # xq is st-major: [p, st, dt, c] with q = st*128 + c
        xq_v = xq_sb[:].rearrange("p (st dt c) -> p st dt c", st=NQT, dt=NDT)

        # --- constants (emitted after the startup-critical DMAs) ---
        r_sb = consts.tile([P, NDT], F32)
        nc.sync.dma_start(out=r_sb[:], in_=r_d[:])
        bv_sb = consts.tile([P, D], F32)
        nc.gpsimd.dma_start(out=bv_sb[:], in_=bv_d[:].to_broadcast([P, D]))
        ones_sb = consts.tile([P, 8], BF16)
        nc.vector.memset(ones_sb[:], 1.0)

        # --- phase V: each core projects only its OWN sequence half of V,
        # then the core pair exchanges halves via AllGather. Gathered order
        # is [half0 | half1] on both cores = natural sequence order, which
        # matches the key order of xt.
        pairs = [[2 * i, 2 * i + 1] for i in range(NCORES // 2)]

        kv_v = dpool.tile([P, NST // 2, D], BF16, tag="kv_v")
        kv_vo = dpool.tile([2, P, NST // 2, D], BF16, tag="kv_vo")

        v_sb = proj.tile([P, NST * D], BF16)  # V[s, e] full, s-tile major
        v_view = v_sb[:].rearrange("p (t e) -> p t e", t=NST)

        # ec0 chains for every st first (they only need wv-ec0 + the st'th xq
        # block), then ec1 chains: the ec1 weight chunk and the tail of xq
        # stream in under the ec0 compute
        # dt-outer over st-pairs: MM (dt, st) only needs the st'th 256KB xq
        # piece + the (dt//2)'th 256KB wv piece, so compute ramps with the
        # earliest DMA arrivals instead of waiting for a full 1MB+ chunk
        vown = stage.tile([P, (NST // 2) * D], BF16)
        for ec in range(NEC):
            for sp in range(NST // 4):
                psum_st = [ps.tile([P, 512], F32, name="psum") for _ in range(2)]
                for dt in range(NDT):
                    for k in range(2):
                        nc.tensor.matmul(
                            psum_st[k][:],
                            lhsT=xq_v[:, 2 * sp + k, dt, :],
                            rhs=wv_sb[
                                :, ec * NDT * 512 + dt * 512: ec * NDT * 512 + dt * 512 + 512
                            ],
                            start=(dt == 0),
                            stop=(dt == NDT - 1),
                        )
                for k in range(2):
                    st = 2 * sp + k
                    nc.scalar.copy(
                        vown[:, st * D + ec * 512: st * D + ec * 512 + 512],
                        psum_st[k][:],
                    )
                    if ec == 1:
                        # export on the scalar queue: sync/gpsimd still carry
                        # input triggers, which would delay the staging ring
                        nc.scalar.dma_start(
                            out=kv_v[:, st, :], in_=vown[:, st * D:(st + 1) * D]
                        )
                if ec == 0 and sp == 1:
                    nc.scalar.dma_start(out=a_sb[:], in_=a_d[:])
                if ec == 0 and sp == 3:
                    nc.scalar.dma_start(out=xt_sb[:], in_=xt_d[:])

        nc.gpsimd.collective_compute(
            "AllGather", mybir.AluOpType.bypass, replica_groups=pairs,
            ins=[kv_v[:]], outs=[kv_vo[:]],
        )
        for r in range(2):
            trig[r % 2].dma_start(
                out=v_view[:, (NST // 2) * r:(NST // 2) * (r + 1), :], in_=kv_vo[r]
            )

        # --- phase P: Pt[d, q] = sum_d' A[d', d] xt[d', q] (+ r via bias)
        pt_sb = proj.tile([P, NDT * SQ], BF16)  # Pt[d, q], d-tile major
        for dto in range(NDT):
            psum_qc = [ps.tile([P, 512], F32, name="psum") for q in range(NQC)]
            for dt in range(NDT):
                for qc in range(NQC):
                    nc.tensor.matmul(
                        psum_qc[qc][:],
                        lhsT=a_sb[:, dt * D + dto * P: dt * D + (dto + 1) * P],
                        rhs=xq_v[:, qc * 4:(qc + 1) * 4, dt, :],
                        start=(dt == 0),
                        stop=(dt == NDT - 1),
                    )
            for qc in range(NQC):
                nc.scalar.activation(
                    pt_sb[:, dto * SQ + qc * 512: dto * SQ + qc * 512 + 512],
                    psum_qc[qc][:], AF.Identity,
                    bias=r_sb[:, dto:dto + 1], scale=1.0,
                )

        # --- phase B: attention, per 512-query chunk ---
        for qc in range(NQC):
            # scoresT[k, q] for all 2048 keys; exp lands in SBUF in exactly
            # the layout attn@V needs as lhsT (no transposes)
            attnT = epool.tile([P, NST * 512], BF16, tag=f"attnT{qc % 2}")
            for ks in range(NST):
                psum = ps.tile([P, 512], F32, name="psum")
                for dt in range(NDT):
                    nc.tensor.matmul(
                        psum[:],
                        lhsT=xt_sb[:, dt * S + ks * P: dt * S + (ks + 1) * P],
                        rhs=pt_sb[:, dt * SQ + qc * 512: dt * SQ + qc * 512 + 512],
                        start=(dt == 0),
                        stop=(dt == NDT - 1),
                    )
                nc.scalar.activation(
                    attnT[:, ks * 512:(ks + 1) * 512], psum[:],
                    AF.Exp, bias=0.0, scale=SCALE,
                )
            for qtl in range(4):
                qt = qc * 4 + qtl
                q0 = qtl * P
                last = qt == NQT - 1
                den_ps = pst.tile([P, 8], F32, name="den")
                psum_ec = [ps.tile([P, 512], F32, name="psum") for e in range(NEC)]
                recip = bpool.tile([P, 1], F32, tag="recip", name="recip")
                out_sb = bpool.tile([P, D], F32, tag=f"osb{qt % 2}", name="osb")

                def drain(ec):
                    sl = slice(ec * 512, (ec + 1) * 512)
                    nc.scalar.activation(
                        out_sb[:, sl], psum_ec[ec][:], AF.Identity,
                        bias=0.0, scale=recip[:],
                    )
                    nc.vector.tensor_add(out_sb[:, sl], out_sb[:, sl], bv_sb[:, sl])
                    nc.sync.dma_start(
                        out=out_d[qt * P:(qt + 1) * P, ec * 512:(ec + 1) * 512],
                        in_=out_sb[:, sl],
                    )

                if not last:
                    # interleaved: one LDWEIGHTS' worth of weight-port traffic
                    # serves the den + both ec chains per key tile
                    for ks in range(NST):
                        lhsT = attnT[:, ks * 512 + q0: ks * 512 + q0 + P]
                        nc.tensor.matmul(
                            den_ps[:], lhsT=lhsT, rhs=ones_sb[:],
                            start=(ks == 0), stop=(ks == NST - 1),
                        )
                        for ec in range(NEC):
                            nc.tensor.matmul(
                                psum_ec[ec][:],
                                lhsT=lhsT,
                                rhs=v_view[:, ks, ec * 512:(ec + 1) * 512],
                                start=(ks == 0), stop=(ks == NST - 1),
                            )
                    nc.vector.reciprocal(recip[:], den_ps[:, 0:1])
                    for ec in range(NEC):
                        drain(ec)
                else:
                    # final tile: sequential chains so the ec0 drain + bias +
                    # store overlap the ec1 chain instead of trailing the
                    # very last matmul
                    for ks in range(NST):
                        lhsT = attnT[:, ks * 512 + q0: ks * 512 + q0 + P]
                        nc.tensor.matmul(
                            den_ps[:], lhsT=lhsT, rhs=ones_sb[:],
                            start=(ks == 0), stop=(ks == NST - 1),
                        )
                        nc.tensor.matmul(
                            psum_ec[0][:], lhsT=lhsT, rhs=v_view[:, ks, 0:512],
                            start=(ks == 0), stop=(ks == NST - 1),
                        )
                    nc.vector.reciprocal(recip[:], den_ps[:, 0:1])
                    for ks in range(NST):
                        nc.tensor.matmul(
                            psum_ec[1][:],
                            lhsT=attnT[:, ks * 512 + q0: ks * 512 + q0 + P],
                            rhs=v_view[:, ks, 512:1024],
                            start=(ks == 0), stop=(ks == NST - 1),
                        )
                        if ks == 1:
                            drain(0)
                    drain(1)

    nc.compile()
    return nc


def get_program():
    global _PROGRAM
    if _PROGRAM is None:
        _PROGRAM = _build_program()
    return _PROGRAM


def make_in_maps(x, Wq, bq, Wk, bk, Wv, bv):
    """Host-side sharding/layout/weight prep. bk is intentionally unused
    (softmax shift invariance along the key axis)."""
    x = np.asarray(x, dtype=np.float32)
    Wq = np.asarray(Wq, dtype=np.float32)
    Wk = np.asarray(Wk, dtype=np.float32)

    # scores folding: A = Wq^T @ Wk (fp32 on host), r = bq @ Wk
    a_t = np.ascontiguousarray((Wq.T @ Wk).astype(NPBF16))
    r_vec = (np.asarray(bq, dtype=np.float32) @ Wk).astype(np.float32)
    r2 = np.ascontiguousarray(r_vec.reshape(NDT, P).T)

    # wv packed [p, ec, dt, 512]: out[p, ec, dt, j] = Wv.T[dt*128+p, ec*512+j]
    wvT = np.asarray(Wv, dtype=np.float32).T.astype(NPBF16)
    wv_t = np.ascontiguousarray(
        wvT.reshape(NDT, P, NEC, 512).transpose(1, 2, 0, 3).reshape(P, NEC * NDT * 512)
    )
    # A packed [p, dt, d]: a2[p, dt, j] = A[dt*128+p, j]
    a2 = np.ascontiguousarray(
        a_t.reshape(NDT, P, D).transpose(1, 0, 2).reshape(P, NDT * D)
    )
    bv2 = np.asarray(bv, dtype=np.float32).reshape(1, D)

    in_maps = []
    xts = [np.ascontiguousarray(x[b].T.astype(NPBF16)) for b in range(B)]
    # xt packed [p, dt, s]; keys in natural order (= gathered V row order)
    xt2s = [
        np.ascontiguousarray(xt.reshape(NDT, P, S).transpose(1, 0, 2).reshape(P, NDT * S))
        for xt in xts
    ]
    for c in range(NCORES):
        b, h = divmod(c, 2)
        xqT = xts[b][:, h * SQ:(h + 1) * SQ]
        # xq packed st-major [p, st, dt, c] with q = st*128 + c: V's (st, ec0)
        # chains can start after only the st'th 256KB block lands
        xq2 = np.ascontiguousarray(
            xqT.reshape(NDT, P, NQT, P).transpose(1, 2, 0, 3).reshape(P, NQT * NDT * P)
        )
        in_maps.append({
            "xt": xt2s[b],
            "xq": xq2,
            "a": a2, "wv": wv_t, "r": r2, "bv": bv2,
        })
    return in_maps


def assemble(results):
    out = np.empty((B, S, D), dtype=np.float32)
    for c in range(NCORES):
        b, h = divmod(c, 2)
        out[b, h * SQ:(h + 1) * SQ, :] = results[c]["out"]
    return out


def kernel(x, Wq, bq, Wk, bk, Wv, bv, _trace=False, _trace_kwargs=None):
    from concourse.bass_utils import run_bass_kernel_spmd

    nc = get_program()
    in_maps = make_in_maps(x, Wq, bq, Wk, bk, Wv, bv)
    res = run_bass_kernel_spmd(
        nc, in_maps, list(range(NCORES)), trace=_trace, **(_trace_kwargs or {})
    )
    out = assemble(res.results)
    if _trace:
        kernel.last_results = res
    return out


# revision 37
# speedup vs baseline: 1.0101x; 1.0101x over previous
"""Single-head attention (B=4, S=2048, D=1024) on 8 Trainium2 NeuronCores.

Sharding: core c handles batch b = c//2, query half h = c%2 (1024 queries).
V for the full sequence is obtained by each core projecting its own half and
exchanging halves with its pair via AllGather.

Math notes (exact rewrites of the reference):
  - scores = (x@Wq^T + bq)(x@Wk^T + bk)^T / 32. Softmax is invariant to
    per-row constants, so the bk terms drop. The rest factors as
      scores = (x @ A + r) @ x^T / 32,  A = Wq^T @ Wk,  r = bq @ Wk,
    with A, r precomputed on host in fp32 (weight prep). This removes the
    K projection from the device entirely.
  - attn rows sum to 1, so the V bias bv is a constant additive term on
    the output: out = attn @ V_nobias + bv.
  - softmax is computed without max-subtraction: |scores/32| < ~4 for
    this problem (checked host-side), exp() is well-conditioned there.

Device pipeline per core (all matmul operands bf16, PSUM accumulation fp32):
  Phase V:  V[s,e] = xq^T @ WvT for own half (PE), exported per s-tile,
            pair AllGather -> full V in SBUF.
  Phase P:  Pt[d,q] = A^T-applied projection (+r via ACT bias) -> bf16.
  Phase B:  per 512-query chunk: scoresT[k,q] = xt^T(slice) @ Pt (PE),
            exp(s/32) on ACT straight into SBUF (this IS the attn@V lhsT —
            no transposes); per 128-query tile: den[q] via ones-matmul
            accumulation, out accум = attnT^T @ V; ACT applies 1/den,
            DVE adds bv; DMA out.
"""

import numpy as np
import ml_dtypes

from contextlib import ExitStack

import concourse.bass as bass
import concourse.mybir as mybir
import concourse.tile as tile
from concourse import bacc

BF16 = mybir.dt.bfloat16
F32 = mybir.dt.float32
NPBF16 = ml_dtypes.bfloat16

B, S, D = 4, 2048, 1024
NCORES = 8
SQ = S // 2            # queries per core
P = 128                # partitions
NDT = D // P           # 8 d-tiles (feature dim)
NST = S // P           # 16 key tiles
NQT = SQ // P          # 8 query tiles per core
NQC = SQ // 512        # 2 query chunks of 512
NEC = D // 512         # 2 embed chunks of 512
SCALE = 1.0 / 32.0     # 1/sqrt(D)

AF = mybir.ActivationFunctionType

_PROGRAM = None


def _build_program():
    nc = bacc.Bacc(
        "TRN2", target_bir_lowering=False, debug=False, num_devices=NCORES
    )
    # all inputs host-packed partition-tiled [128, n]: one fully-contiguous
    # (16KB+ rows) DMA each — trigger issue (~700ns each) and small-packet
    # overhead dominated the startup otherwise
    xt_d = nc.dram_tensor("xt", [P, NDT * S], BF16, kind="ExternalInput")
    xq_d = nc.dram_tensor("xq", [P, NQT * NDT * P], BF16, kind="ExternalInput")
    a_d = nc.dram_tensor("a", [P, NDT * D], BF16, kind="ExternalInput")
    wv_d = nc.dram_tensor("wv", [P, NEC * NDT * 512], BF16, kind="ExternalInput")
    r_d = nc.dram_tensor("r", [P, NDT], F32, kind="ExternalInput")
    bv_d = nc.dram_tensor("bv", [1, D], F32, kind="ExternalInput")
    out_d = nc.dram_tensor("out", [SQ, D], F32, kind="ExternalOutput")

    with tile.TileContext(nc) as tc, ExitStack() as ctx:
        consts = ctx.enter_context(tc.tile_pool(name="consts", bufs=1))
        xpool = ctx.enter_context(tc.tile_pool(name="xpool", bufs=1))
        wpool = ctx.enter_context(tc.tile_pool(name="wpool", bufs=1))
        stage = ctx.enter_context(tc.tile_pool(name="stage", bufs=1))
        proj = ctx.enter_context(tc.tile_pool(name="proj", bufs=1))
        epool = ctx.enter_context(tc.tile_pool(name="epool", bufs=1))
        bpool = ctx.enter_context(tc.tile_pool(name="bpool", bufs=1))
        dpool = ctx.enter_context(tc.tile_pool(name="dpool", bufs=1, space="DRAM"))
        ps = ctx.enter_context(tc.tile_pool(name="ps", bufs=5, space="PSUM"))
        pst = ctx.enter_context(tc.tile_pool(name="pst", bufs=3, space="PSUM"))

        # --- PE warm-up: dummy matmuls on a zeroed tile keep the PE busy
        # (and the HAM clock-gate warming) while the first inputs land ---
        warm = consts.tile([P, 640], BF16)
        nc.vector.memset(warm[:], 0.0)

        # tiny warm-up collective: absorbs any one-time CC channel setup
        # latency before the real V exchange
        ccw_in = dpool.tile([1, 256], BF16, tag="ccw_in")
        ccw_out = dpool.tile([2, 256], BF16, tag="ccw_out")
        nc.gpsimd.dma_start(out=ccw_in[:], in_=warm[0:1, 0:256])
        nc.gpsimd.collective_compute(
            "AllGather", mybir.AluOpType.bypass,
            replica_groups=[[2 * i, 2 * i + 1] for i in range(NCORES // 2)],
            ins=[ccw_in[:]], outs=[ccw_out[:]],
        )
        for _ in range(14):
            wps = ps.tile([P, 512], F32, name="psum")
            nc.tensor.matmul(
                wps[:], lhsT=warm[:, 512:640], rhs=warm[:, 0:512],
                start=True, stop=True,
            )

        # --- input loads: DMA trigger issue costs ~700ns per dma_start on
        # the issuing engine's queue, so round-robin the triggers over the
        # HWDGE-capable engines, first-needed data first ---
        # keep the scalar engine OUT of trigger duty: it must drain the
        # first V PSUMs while inputs are still streaming
        trig = [nc.sync, nc.gpsimd]
        _t = [0]

        def dma(out, in_):
            trig[_t[0] % len(trig)].dma_start(out=out, in_=in_)
            _t[0] += 1

        # first-needed first: V(st, ec0) chains need wv-ec0 + the st'th 256KB
        # block of the st-major-packed xq. DMA-completion semaphores are per
        # dma_start, so xq is split into 2-st-block pieces: the first V chain
        # only waits on wv-ec0 + piece 0, not the whole 2MB
        # interleave wv-ec0 (2-dt 256KB pieces) with per-st 256KB xq pieces:
        # the first V matmul needs only wv piece 0 + xq piece 0 = 512KB
        wv_sb = wpool.tile([P, NEC * NDT * 512], BF16, tag="w")
        xq_sb = xpool.tile([P, NQT * NDT * P], BF16)
        XB = NQT * NDT * P // 8
        dma(wv_sb[:, 0:1024], wv_d[:, 0:1024])
        dma(xq_sb[:, 0:XB], xq_d[:, 0:XB])
        for b_ in range(1, 4):
            dma(wv_sb[:, b_ * 1024:(b_ + 1) * 1024], wv_d[:, b_ * 1024:(b_ + 1) * 1024])
            dma(xq_sb[:, b_ * XB:(b_ + 1) * XB], xq_d[:, b_ * XB:(b_ + 1) * XB])
        for b_ in range(4, 8):
            dma(xq_sb[:, b_ * XB:(b_ + 1) * XB], xq_d[:, b_ * XB:(b_ + 1) * XB])
        dma(wv_sb[:, NDT * 512:2 * NDT * 512], wv_d[:, NDT * 512:2 * NDT * 512])
        # a/xt are triggered from the scalar queue mid-V-phase (below): the
        # DMA queue shares bandwidth round-robin across active transfers, so
        # issuing them now would delay the startup-critical wv/xq pieces
        a_sb = xpool.tile([P, NDT * D], BF16)
        xt_sb = xpool.tile([P, NDT * S], BF16)
        # xq is st-major: [p, st, dt, c] with q = st*128 + c
        xq_v = xq_sb[:].rearrange("p (st dt c) -> p st dt c", st=NQT, dt=NDT)

        # --- constants (emitted after the startup-critical DMAs) ---
        r_sb = consts.tile([P, NDT], F32)
        nc.sync.dma_start(out=r_sb[:], in_=r_d[:])
        bv_sb = consts.tile([P, D], F32)
        nc.gpsimd.dma_start(out=bv_sb[:], in_=bv_d[:].to_broadcast([P, D]))
        ones_sb = consts.tile([P, 8], BF16)
        nc.vector.memset(ones_sb[:], 1.0)

        # --- phase V: each core projects only its OWN sequence half of V,
        # then the core pair exchanges halves via AllGather. Gathered order
        # is [half0 | half1] on both cores = natural sequence order, which
        # matches the key order of xt.
        pairs = [[2 * i, 2 * i + 1] for i in range(NCORES // 2)]

        kv_v = dpool.tile([P, NST // 2, D], BF16, tag="kv_v")
        kv_vo = dpool.tile([2, P, NST // 2, D], BF16, tag="kv_vo")

        v_sb = proj.tile([P, NST * D], BF16)  # V[s, e] full, s-tile major
        v_view = v_sb[:].rearrange("p (t e) -> p t e", t=NST)

        # ec0 chains for every st first (they only need wv-ec0 + the st'th xq
        # block), then ec1 chains: the ec1 weight chunk and the tail of xq
        # stream in under the ec0 compute
        # dt-outer over st-pairs: MM (dt, st) only needs the st'th 256KB xq
        # piece + the (dt//2)'th 256KB wv piece, so compute ramps with the
        # earliest DMA arrivals instead of waiting for a full 1MB+ chunk
        vown = stage.tile([P, (NST // 2) * D], BF16)
        for ec in range(NEC):
            for sp in range(NST // 4):
                psum_st = [ps.tile([P, 512], F32, name="psum") for _ in range(2)]
                for dt in range(NDT):
                    for k in range(2):
                        nc.tensor.matmul(
                            psum_st[k][:],
                            lhsT=xq_v[:, 2 * sp + k, dt, :],
                            rhs=wv_sb[
                                :, ec * NDT * 512 + dt * 512: ec * NDT * 512 + dt * 512 + 512
                            ],
                            start=(dt == 0),
                            stop=(dt == NDT - 1),
                        )
                for k in range(2):
                    st = 2 * sp + k
                    nc.scalar.copy(
                        vown[:, st * D + ec * 512: st * D + ec * 512 + 512],
                        psum_st[k][:],
                    )
                    if ec == 1:
                        # export via sync: its HW queue (shared with gpsimd)
                        # is drained of input pieces by now, while scalar's
                        # queue carries the deferred a/xt streams
                        nc.sync.dma_start(
                            out=kv_v[:, st, :], in_=vown[:, st * D:(st + 1) * D]
                        )
                # a/xt ride scalar's separate HW queue, deferred so they never
                # compete with the startup-critical wv/xq pieces on Q1
                if ec == 1 and sp == 0:
                    nc.scalar.dma_start(out=a_sb[:], in_=a_d[:])
                if ec == 1 and sp == 2:
                    nc.scalar.dma_start(out=xt_sb[:], in_=xt_d[:])

        nc.gpsimd.collective_compute(
            "AllGather", mybir.AluOpType.bypass, replica_groups=pairs,
            ins=[kv_v[:]], outs=[kv_vo[:]],
        )
        for r in range(2):
            trig[r % 2].dma_start(
                out=v_view[:, (NST // 2) * r:(NST // 2) * (r + 1), :], in_=kv_vo[r]
            )

        # --- phase P: Pt[d, q] = sum_d' A[d', d] xt[d', q] (+ r via bias)
        pt_sb = proj.tile([P, NDT * SQ], BF16)  # Pt[d, q], d-tile major
        for dto in range(NDT):
            psum_qc = [ps.tile([P, 512], F32, name="psum") for q in range(NQC)]
            for dt in range(NDT):
                for qc in range(NQC):
                    nc.tensor.matmul(
                        psum_qc[qc][:],
                        lhsT=a_sb[:, dt * D + dto * P: dt * D + (dto + 1) * P],
                        rhs=xq_v[:, qc * 4:(qc + 1) * 4, dt, :],
                        start=(dt == 0),
                        stop=(dt == NDT - 1),
                    )
            for qc in range(NQC):
                nc.scalar.activation(
                    pt_sb[:, dto * SQ + qc * 512: dto * SQ + qc * 512 + 512],
                    psum_qc[qc][:], AF.Identity,
                    bias=r_sb[:, dto:dto + 1], scale=1.0,
                )

        # --- phase B: attention, per 512-query chunk ---
        for qc in range(NQC):
            # scoresT[k, q] for all 2048 keys; exp lands in SBUF in exactly
            # the layout attn@V needs as lhsT (no transposes)
            attnT = epool.tile([P, NST * 512], BF16, tag=f"attnT{qc % 2}")
            for ks in range(NST):
                psum = ps.tile([P, 512], F32, name="psum")
                for dt in range(NDT):
                    nc.tensor.matmul(
                        psum[:],
                        lhsT=xt_sb[:, dt * S + ks * P: dt * S + (ks + 1) * P],
                        rhs=pt_sb[:, dt * SQ + qc * 512: dt * SQ + qc * 512 + 512],
                        start=(dt == 0),
                        stop=(dt == NDT - 1),
                    )
                nc.scalar.activation(
                    attnT[:, ks * 512:(ks + 1) * 512], psum[:],
                    AF.Exp, bias=0.0, scale=SCALE,
                )
            for qtl in range(4):
                qt = qc * 4 + qtl
                q0 = qtl * P
                last = qt == NQT - 1
                den_ps = pst.tile([P, 8], F32, name="den")
                psum_ec = [ps.tile([P, 512], F32, name="psum") for e in range(NEC)]
                recip = bpool.tile([P, 1], F32, tag="recip", name="recip")
                out_sb = bpool.tile([P, D], F32, tag=f"osb{qt % 2}", name="osb")

                def drain(ec):
                    sl = slice(ec * 512, (ec + 1) * 512)
                    nc.scalar.activation(
                        out_sb[:, sl], psum_ec[ec][:], AF.Identity,
                        bias=0.0, scale=recip[:],
                    )
                    nc.vector.tensor_add(out_sb[:, sl], out_sb[:, sl], bv_sb[:, sl])
                    nc.sync.dma_start(
                        out=out_d[qt * P:(qt + 1) * P, ec * 512:(ec + 1) * 512],
                        in_=out_sb[:, sl],
                    )

                if not last:
                    # interleaved: one LDWEIGHTS' worth of weight-port traffic
                    # serves the den + both ec chains per key tile
                    for ks in range(NST):
                        lhsT = attnT[:, ks * 512 + q0: ks * 512 + q0 + P]
                        nc.tensor.matmul(
                            den_ps[:], lhsT=lhsT, rhs=ones_sb[:],
                            start=(ks == 0), stop=(ks == NST - 1),
                        )
                        for ec in range(NEC):
                            nc.tensor.matmul(
                                psum_ec[ec][:],
                                lhsT=lhsT,
                                rhs=v_view[:, ks, ec * 512:(ec + 1) * 512],
                                start=(ks == 0), stop=(ks == NST - 1),
                            )
                    nc.vector.reciprocal(recip[:], den_ps[:, 0:1])
                    for ec in range(NEC):
                        drain(ec)
                else:
                    # final tile: sequential chains so the ec0 drain + bias +
                    # store overlap the ec1 chain instead of trailing the
                    # very last matmul
                    for ks in range(NST):
                        lhsT = attnT[:, ks * 512 + q0: ks * 512 + q0 + P]
                        nc.tensor.matmul(
                            den_ps[:], lhsT=lhsT, rhs=ones_sb[:],
                            start=(ks == 0), stop=(ks == NST - 1),
                        )
                        nc.tensor.matmul(
                            psum_ec[0][:], lhsT=lhsT, rhs=v_view[:, ks, 0:512],
                            start=(ks == 0), stop=(ks == NST - 1),
                        )
                    nc.vector.reciprocal(recip[:], den_ps[:, 0:1])
                    for ks in range(NST):
                        nc.tensor.matmul(
                            psum_ec[1][:],
                            lhsT=attnT[:, ks * 512 + q0: ks * 512 + q0 + P],
                            rhs=v_view[:, ks, 512:1024],
                            start=(ks == 0), stop=(ks == NST - 1),
                        )
                        if ks == 1:
                            drain(0)
                    drain(1)

    nc.compile()
    return nc


def get_program():
    global _PROGRAM
    if _PROGRAM is None:
        _PROGRAM = _build_program()
    return _PROGRAM


def make_in_maps(x, Wq, bq, Wk, bk, Wv, bv):
    """Host-side sharding/layout/weight prep. bk is intentionally unused
    (softmax shift invariance along the key axis)."""
    x = np.asarray(x, dtype=np.float32)
    Wq = np.asarray(Wq, dtype=np.float32)
    Wk = np.asarray(Wk, dtype=np.float32)

    # scores folding: A = Wq^T @ Wk (fp32 on host), r = bq @ Wk
    a_t = np.ascontiguousarray((Wq.T @ Wk).astype(NPBF16))
    r_vec = (np.asarray(bq, dtype=np.float32) @ Wk).astype(np.float32)
    r2 = np.ascontiguousarray(r_vec.reshape(NDT, P).T)

    # wv packed [p, ec, dt, 512]: out[p, ec, dt, j] = Wv.T[dt*128+p, ec*512+j]
    wvT = np.asarray(Wv, dtype=np.float32).T.astype(NPBF16)
    wv_t = np.ascontiguousarray(
        wvT.reshape(NDT, P, NEC, 512).transpose(1, 2, 0, 3).reshape(P, NEC * NDT * 512)
    )
    # A packed [p, dt, d]: a2[p, dt, j] = A[dt*128+p, j]
    a2 = np.ascontiguousarray(
        a_t.reshape(NDT, P, D).transpose(1, 0, 2).reshape(P, NDT * D)
    )
    bv2 = np.asarray(bv, dtype=np.float32).reshape(1, D)

    in_maps = []
    xts = [np.ascontiguousarray(x[b].T.astype(NPBF16)) for b in range(B)]
    # xt packed [p, dt, s]; keys in natural order (= gathered V row order)
    xt2s = [
        np.ascontiguousarray(xt.reshape(NDT, P, S).transpose(1, 0, 2).reshape(P, NDT * S))
        for xt in xts
    ]
    for c in range(NCORES):
        b, h = divmod(c, 2)
        xqT = xts[b][:, h * SQ:(h + 1) * SQ]
        # xq packed st-major [p, st, dt, c] with q = st*128 + c: V's (st, ec0)
        # chains can start after only the st'th 256KB block lands
        xq2 = np.ascontiguousarray(
            xqT.reshape(NDT, P, NQT, P).transpose(1, 2, 0, 3).reshape(P, NQT * NDT * P)
        )
        in_maps.append({
            "xt": xt2s[b],
            "xq": xq2,
            "a": a2, "wv": wv_t, "r": r2, "bv": bv2,
        })
    return in_maps


def assemble(results):
    out = np.empty((B, S, D), dtype=np.float32)
    for c in range(NCORES):
        b, h = divmod(c, 2)
        out[b, h * SQ:(h + 1) * SQ, :] = results[c]["out"]
    return out


def kernel(x, Wq, bq, Wk, bk, Wv, bv, _trace=False, _trace_kwargs=None):
    from concourse.bass_utils import run_bass_kernel_spmd

    nc = get_program()
    in_maps = make_in_maps(x, Wq, bq, Wk, bk, Wv, bv)
    res = run_bass_kernel_spmd(
        nc, in_maps, list(range(NCORES)), trace=_trace, **(_trace_kwargs or {})
    )
    out = assemble(res.results)
    if _trace:
        kernel.last_results = res
    return out
